# revision 1
# baseline (speedup 1.0000x reference)
"""DAG additive-attention kernel for 8 Trainium2 NeuronCores.

Reference computation (single fp32 graph):
    x = concat([leaves, ancestors], -1)            # [N, 2048]
    h = tanh(x @ W1 + b1)                          # [N, 512]
    scores = h @ W2 + b2                           # [N, 1]
    attn = softmax(scores, axis=0)
    out = attn.squeeze(1) @ ancestors              # [1024]

Distribution: shard N across the 8 cores. Softmax over N needs no on-device
collective: b2 is a constant shift (softmax-invariant, dropped), scores are
bounded (|h| <= 1 so |s| <= sum|W2| ~ 8), so exp() without max subtraction is
safe in fp32. Each core returns (sum_i exp(s_i) * ancestors_i, sum_i exp(s_i))
over its shard and the host combines in fp64.

Device pipeline per core (R = 8192 rows), processed in 512-row slabs whose
DMA reads are fully contiguous thanks to host-side pre-tiling: for each
128-row tile, 16 K-chunk bf16 matmuls (stationary = transposed x chunk,
moving = W1 chunk) accumulate h in PSUM fp32; DVE adds b1, ACT applies tanh,
DVE (*W2, reduce) and ACT exp produce the tile's softmax numerators, written
into a per-slab [128, 4] grid that then becomes the stationary operand of
matmuls against the natural-layout ancestors tile, accumulated into
persistent PSUM accumulators (plus one tiny matmul per slab against a ones
vector for the denominator). The step-3 matmul group trails the main matmul
stream by one slab so the PE never stalls on the score chain, and the PE
clock gate is pre-warmed with dummy matmuls during the initial DMA window.
"""

import sys

for _p in ("/opt/trn_rl_repo", "/opt/pypackages"):
    if _p not in sys.path:
        sys.path.append(_p)

from concurrent.futures import ThreadPoolExecutor
from contextlib import ExitStack

import ml_dtypes
import numpy as np

import concourse.tile as tile
from concourse import bacc, mybir
from concourse.bass import ts
from concourse.bass_utils import run_bass_kernel_spmd

N, EMB, ATT = 65536, 1024, 512
N_CORES = 8
R = N // N_CORES          # rows per core
KF = 2 * EMB              # contraction (feature) dim
KC = KF // 128            # k-chunks of 128
SLAB = 512                # rows per DMA slab
NSLAB = R // SLAB
SUB = SLAB // 128         # 128-row subtiles per slab
NT = R // 128             # row tiles per core
BF16 = mybir.dt.bfloat16
F32 = mybir.dt.float32

LAST_RESULTS = None       # BassKernelResults of the most recent run


WARM_MMS = 44  # PE clock-gate warmup matmuls emitted before the main loop


def _kernel_body(ctx, tc, xT, anc, w1, b1bc, w2b, partial_d, wx_out_d):
    nc = tc.nc
    singles = ctx.enter_context(tc.tile_pool(name="singles", bufs=1))
    xt_pool = ctx.enter_context(tc.tile_pool(name="xt", bufs=4))
    anc_pool = ctx.enter_context(tc.tile_pool(name="anc", bufs=5))
    hb_pool = ctx.enter_context(tc.tile_pool(name="hb", bufs=3))
    th_pool = ctx.enter_context(tc.tile_pool(name="th", bufs=3))
    sc_pool = ctx.enter_context(tc.tile_pool(name="sc", bufs=4))
    wx_pool = ctx.enter_context(tc.tile_pool(name="wx", bufs=3))
    h_pool = ctx.enter_context(tc.tile_pool(name="hps", bufs=4, space="PSUM"))
    acc_pool = ctx.enter_context(tc.tile_pool(name="acc", bufs=1, space="PSUM"))
    out_pool = ctx.enter_context(tc.tile_pool(name="outs", bufs=1))

    # Weights / constants, resident for the whole kernel. Issued on the
    # scalar-engine HWDGE ring so they overlap the first xT slab (sync ring).
    # All DRAM operands arrive pre-tiled from the host so every transfer
    # below is a fully contiguous read.
    w1_sb = singles.tile([128, KC, ATT], BF16)
    for q in range(4):
        nc.scalar.dma_start(w1_sb[:, ts(q, KC // 4), :], w1[:, ts(q, KC // 4), :])
    b1bc_sb = singles.tile([128, ATT], BF16)
    nc.scalar.dma_start(b1bc_sb[:], b1bc[:])
    w2b_sb = singles.tile([128, ATT], F32)
    nc.scalar.dma_start(w2b_sb[:], w2b[:])
    ones_mv = singles.tile([128, 1], BF16)
    nc.vector.memset(ones_mv[:], 1.0)

    # Warm the PE clock gate during the initial DMA window: dependency-free
    # matmuls on a zeroed tile keep TensorE busy >3.4us so the HAM throttle is
    # released right around the time the first slab and weights arrive.
    if WARM_MMS:
        warm_sb = singles.tile([128, 128], BF16)
        nc.vector.memset(warm_sb[:], 0.0)
        warm_ps = h_pool.tile([128, 128], F32, tag="h")
        for _ in range(WARM_MMS):
            nc.tensor.matmul(
                warm_ps[:], warm_sb[:], warm_sb[:], start=True, stop=True
            )

    # Persistent PSUM accumulators: weighted ancestor sums (acc0/acc1) and the
    # per-subtile-phase sums of the exp weights (acc_se, reduced on host).
    acc0 = acc_pool.tile([1, ATT], F32, tag="acc0")
    acc1 = acc_pool.tile([1, ATT], F32, tag="acc1")
    acc_se = acc_pool.tile([SUB, 1], F32, tag="accse")

    def emit_step3_tile(wx4, an_t, s, u):
        t = s * SUB + u
        st, sp = (t == 0), (t == NT - 1)
        nc.tensor.matmul(acc0[:], wx4[:, u : u + 1], an_t[:, u, 0:ATT],
                         start=st, stop=sp)
        nc.tensor.matmul(acc1[:], wx4[:, u : u + 1], an_t[:, u, ATT:EMB],
                         start=st, stop=sp)

    def emit_step3_slab(p):
        wx4, an_t, s = p
        for u in range(SUB):
            emit_step3_tile(wx4, an_t, s, u)
        nc.tensor.matmul(acc_se[:], wx4[:], ones_mv[:],
                         start=(s == 0), stop=(s == NSLAB - 1))

    pending = None
    for s in range(NSLAB):
        xt = xt_pool.tile([128, KC, SLAB], BF16, tag="xt")
        if s == 0:
            # Split the first slab by k-chunks (contiguous in the tiled
            # layout) so the opening k-loop can start after the first
            # quarter arrives, pipelining through the rest.
            for q in range(4):
                nc.sync.dma_start(
                    xt[:, ts(q, KC // 4), :], xT[s, :, ts(q, KC // 4), :]
                )
        else:
            nc.sync.dma_start(xt[:], xT[s])
        an = anc_pool.tile([128, SUB, EMB], BF16, tag="an")
        nc.sync.dma_start(an[:], anc[s])
        wx4 = wx_pool.tile([128, SUB], BF16, tag="wx")
        last = s == NSLAB - 1
        for u in range(SUB):
            t = s * SUB + u
            h = h_pool.tile([128, ATT], F32, tag="h")
            for k in range(KC):
                nc.tensor.matmul(
                    h[:], xt[:, k, ts(u, 128)], w1_sb[:, k, :],
                    start=(k == 0), stop=(k == KC - 1),
                )
            if u == 2 and pending is not None:
                emit_step3_slab(pending)
            # In the final slab, trail its own step-3 work one tile behind the
            # matmul stream so only the last tile's score chain sits in the
            # kernel tail.
            if last and u >= 2:
                emit_step3_tile(wx4, an, s, u - 2)
            hb = hb_pool.tile([128, ATT], F32, tag="hb")
            nc.vector.tensor_add(hb[:], h[:], b1bc_sb[:])
            th = th_pool.tile([128, ATT], F32, tag="th")
            nc.scalar.activation(th[:], hb[:], mybir.ActivationFunctionType.Tanh)
            nc.vector.tensor_mul(th[:], th[:], w2b_sb[:])
            sc = sc_pool.tile([128, 1], F32, tag="sc")
            nc.vector.reduce_sum(sc[:], th[:], axis=mybir.AxisListType.X)
            nc.scalar.activation(wx4[:, u : u + 1], sc[:],
                                 mybir.ActivationFunctionType.Exp)
        pending = (wx4, an, s)
    wx4, an, s = pending
    emit_step3_tile(wx4, an, s, SUB - 2)
    emit_step3_tile(wx4, an, s, SUB - 1)
    nc.tensor.matmul(acc_se[:], wx4[:], ones_mv[:],
                     start=(s == 0), stop=True)

    out_sb = out_pool.tile([1, EMB], F32)
    se_sb = out_pool.tile([SUB, 1], F32)
    nc.vector.tensor_copy(out_sb[:, 0:ATT], acc0[:])
    nc.vector.tensor_copy(out_sb[:, ATT:EMB], acc1[:])
    nc.vector.tensor_copy(se_sb[:], acc_se[:])
    nc.sync.dma_start(partial_d[:], out_sb[:])
    nc.scalar.dma_start(wx_out_d[:], se_sb[:])


_nc_cache = None


def _get_nc():
    global _nc_cache
    if _nc_cache is None:
        nc = bacc.Bacc(
            "TRN2", target_bir_lowering=False, debug=False, num_devices=N_CORES
        )
        xT = nc.dram_tensor(
            "xT", [NSLAB, 128, KC, SLAB], BF16, kind="ExternalInput"
        ).ap()
        anc = nc.dram_tensor(
            "anc", [NSLAB, 128, SUB, EMB], BF16, kind="ExternalInput"
        ).ap()
        w1 = nc.dram_tensor("w1", [128, KC, ATT], BF16, kind="ExternalInput").ap()
        b1bc = nc.dram_tensor("b1bc", [128, ATT], BF16, kind="ExternalInput").ap()
        w2b = nc.dram_tensor("w2b", [128, ATT], F32, kind="ExternalInput").ap()
        partial = nc.dram_tensor("partial", [1, EMB], F32, kind="ExternalOutput").ap()
        wx_out = nc.dram_tensor("wx_out", [SUB, 1], F32, kind="ExternalOutput").ap()
        with tile.TileContext(nc) as tc, ExitStack() as ctx:
            _kernel_body(ctx, tc, xT, anc, w1, b1bc, w2b, partial, wx_out)
        nc.compile()
        _nc_cache = nc
    return _nc_cache


def _prep_core(c, leaves, ancestors, shared):
    bf = ml_dtypes.bfloat16
    sl = slice(c * R, (c + 1) * R)
    EC = EMB // 128  # feature chunks per source tensor
    xT = np.empty((NSLAB, 128, KC, SLAB), dtype=bf)
    at = np.empty((NSLAB, 128, SUB, EMB), dtype=bf)
    for s in range(NSLAB):
        rs = slice(c * R + s * SLAB, c * R + (s + 1) * SLAB)
        lv = leaves[rs]      # [SLAB, EMB] fp32
        av = ancestors[rs]
        np.copyto(
            xT[s, :, 0:EC, :], lv.reshape(SLAB, EC, 128).transpose(2, 1, 0),
            casting="unsafe",
        )
        np.copyto(
            xT[s, :, EC:KC, :], av.reshape(SLAB, EC, 128).transpose(2, 1, 0),
            casting="unsafe",
        )
        np.copyto(
            at[s], av.reshape(SUB, 128, EMB).transpose(1, 0, 2), casting="unsafe"
        )
    return {"xT": xT, "anc": at, **shared}


def kernel(leaves, ancestors, W1, b1, W2, b2, *, trace=False):
    global LAST_RESULTS
    nc = _get_nc()
    bf = ml_dtypes.bfloat16
    leaves = np.asarray(leaves, dtype=np.float32)
    ancestors = np.asarray(ancestors, dtype=np.float32)
    shared = {
        "w1": np.ascontiguousarray(
            np.asarray(W1, dtype=np.float32)
            .reshape(KC, 128, ATT)
            .transpose(1, 0, 2)
            .astype(bf)
        ),
        "b1bc": np.ascontiguousarray(
            np.broadcast_to(np.asarray(b1).astype(bf).reshape(1, ATT), (128, ATT))
        ),
        "w2b": np.ascontiguousarray(
            np.broadcast_to(
                np.asarray(W2, dtype=np.float32).reshape(1, ATT), (128, ATT)
            )
        ),
    }
    with ThreadPoolExecutor(max_workers=8) as ex:
        in_maps = list(
            ex.map(lambda c: _prep_core(c, leaves, ancestors, shared), range(N_CORES))
        )
    res = run_bass_kernel_spmd(
        nc, in_maps, core_ids=list(range(N_CORES)), trace=trace
    )
    LAST_RESULTS = res
    num = np.zeros(EMB, dtype=np.float64)
    den = 0.0
    for c in range(N_CORES):
        num += res.results[c]["partial"][0].astype(np.float64)
        den += res.results[c]["wx_out"].astype(np.float64).sum()
    return (num / den).astype(np.float32)



# revision 4
# speedup vs baseline: 1.4889x; 1.4889x over previous
"""DAG additive-attention kernel for 8 Trainium2 NeuronCores.

Reference computation (single fp32 graph):
    x = concat([leaves, ancestors], -1)            # [N, 2048]
    h = tanh(x @ W1 + b1)                          # [N, 512]
    scores = h @ W2 + b2                           # [N, 1]
    attn = softmax(scores, axis=0)
    out = attn.squeeze(1) @ ancestors              # [1024]

Distribution: shard N across the 8 cores. Softmax over N needs no on-device
collective: b2 is a constant shift (softmax-invariant, dropped), scores are
bounded (|h| <= 1 so |s| <= sum|W2| ~ 8), so exp() without max subtraction is
safe in fp32. Each core returns (sum_i exp(s_i) * ancestors_i, sum_i exp(s_i))
over its shard and the host combines in fp64.

The x @ W1 matmul runs in fp8 e4m3 with MatmulPerfMode.DoubleRow (K=256 per
instruction, 2x the bf16 FLOP rate). Precision holds because the dominant
fp8 error term is a bias proportional to (deltaW1_ancestor_rows @ W2) --
quantization noise in W1 correlates with the ancestors through the scores --
so the host quantizes W1*64 with per-row error-feedback rounding that drives
each row's residual projection onto W2 to ~0. The 1/64 descale rides the
(h + b1) DVE op as a scalar multiplier (scalar_tensor_tensor). The W2
multiply + score reduction fuse into one tensor_tensor_reduce DVE op.

Device pipeline per core (R = 8192 rows), processed in 512-row slabs whose
DMA reads are fully contiguous thanks to host-side pre-tiling: for each
128-row tile, 8 k-pair fp8 DoubleRow matmuls (stationary = transposed x pair
chunk, moving = W1 pair chunk, 2 halves of ATT) accumulate h in PSUM fp32;
DVE applies (h/64 + b1), ACT applies tanh, DVE (*W2, reduce) and ACT exp
produce the tile's softmax numerators, written into a per-slab [128, 4] grid
that then becomes the stationary operand of matmuls against the
natural-layout bf16 ancestors tile, accumulated into persistent PSUM
accumulators (plus one tiny matmul per slab against a ones vector for the
denominator). The step-3 matmul group trails the main matmul stream by one
slab so the PE never stalls on the score chain, and the PE clock gate is
pre-warmed with dummy matmuls during the initial DMA window.
"""

import sys

for _p in ("/opt/trn_rl_repo", "/opt/pypackages"):
    if _p not in sys.path:
        sys.path.append(_p)

from concurrent.futures import ThreadPoolExecutor
from contextlib import ExitStack

import ml_dtypes
import numpy as np

import concourse.tile as tile
from concourse import bacc, mybir
from concourse.bass import ts
from concourse.bass_utils import run_bass_kernel_spmd

N, EMB, ATT = 65536, 1024, 512
N_CORES = 8
R = N // N_CORES          # rows per core
KF = 2 * EMB              # contraction (feature) dim
KP = KF // 256            # k-pairs of 256 (fp8 DoubleRow)
SLAB = 512                # rows per DMA slab
NSLAB = R // SLAB
SUB = SLAB // 128         # 128-row subtiles per slab
NT = R // 128             # row tiles per core
BF16 = mybir.dt.bfloat16
F32 = mybir.dt.float32
F8 = mybir.dt.float8e4
E4 = ml_dtypes.float8_e4m3

W1_SCALE = 64.0           # W1 pre-quantization scale (fp8 normal range)

LAST_RESULTS = None       # BassKernelResults of the most recent run


WARM_MMS = 44  # PE clock-gate warmup matmuls emitted before the main loop


def _kernel_body(ctx, tc, xT, anc, w1, b1bc, w2b, partial_d, wx_out_d):
    nc = tc.nc
    DR = mybir.MatmulPerfMode.DoubleRow
    singles = ctx.enter_context(tc.tile_pool(name="singles", bufs=1))
    xt_pool = ctx.enter_context(tc.tile_pool(name="xt", bufs=4))
    anc_pool = ctx.enter_context(tc.tile_pool(name="anc", bufs=5))
    hb_pool = ctx.enter_context(tc.tile_pool(name="hb", bufs=3))
    th_pool = ctx.enter_context(tc.tile_pool(name="th", bufs=3))
    sc_pool = ctx.enter_context(tc.tile_pool(name="sc", bufs=4))
    wx_pool = ctx.enter_context(tc.tile_pool(name="wx", bufs=3))
    h_pool = ctx.enter_context(tc.tile_pool(name="hps", bufs=4, space="PSUM"))
    acc_pool = ctx.enter_context(tc.tile_pool(name="acc", bufs=1, space="PSUM"))
    out_pool = ctx.enter_context(tc.tile_pool(name="outs", bufs=1))

    # Weights / constants, resident for the whole kernel. Issued on the
    # scalar-engine HWDGE ring so they overlap the first xT slab (sync ring).
    # All DRAM operands arrive pre-tiled from the host so every transfer
    # below is a fully contiguous read.
    w1_sb = singles.tile([128, KP, 2, ATT], F8)
    for q in range(4):
        nc.scalar.dma_start(w1_sb[:, ts(q, KP // 4), :, :], w1[:, ts(q, KP // 4), :, :])
    b1bc_sb = singles.tile([128, ATT], BF16)
    nc.scalar.dma_start(b1bc_sb[:], b1bc[:])
    w2b_sb = singles.tile([128, ATT], F32)
    nc.scalar.dma_start(w2b_sb[:], w2b[:])
    ones_mv = singles.tile([128, 1], BF16)
    nc.vector.memset(ones_mv[:], 1.0)

    # Warm the PE clock gate during the initial DMA window: dependency-free
    # matmuls on a zeroed tile keep TensorE busy >3.4us so the HAM throttle is
    # released right around the time the first slab and weights arrive.
    if WARM_MMS:
        warm_sb = singles.tile([128, 128], BF16)
        nc.vector.memset(warm_sb[:], 0.0)
        warm_ps = h_pool.tile([128, 128], F32, tag="h")
        for _ in range(WARM_MMS):
            nc.tensor.matmul(
                warm_ps[:], warm_sb[:], warm_sb[:], start=True, stop=True
            )

    # Persistent PSUM accumulators: weighted ancestor sums (acc0/acc1) and the
    # per-subtile-phase sums of the exp weights (acc_se, reduced on host).
    acc0 = acc_pool.tile([1, ATT], F32, tag="acc0")
    acc1 = acc_pool.tile([1, ATT], F32, tag="acc1")
    acc_se = acc_pool.tile([SUB, 1], F32, tag="accse")

    def emit_step3_tile(wx4, an_t, s, u):
        t = s * SUB + u
        st, sp = (t == 0), (t == NT - 1)
        nc.tensor.matmul(acc0[:], wx4[:, u : u + 1], an_t[:, u, 0:ATT],
                         start=st, stop=sp)
        nc.tensor.matmul(acc1[:], wx4[:, u : u + 1], an_t[:, u, ATT:EMB],
                         start=st, stop=sp)

    def emit_step3_slab(p):
        wx4, an_t, s = p
        for u in range(SUB):
            emit_step3_tile(wx4, an_t, s, u)
        nc.tensor.matmul(acc_se[:], wx4[:], ones_mv[:],
                         start=(s == 0), stop=(s == NSLAB - 1))

    pending = None
    for s in range(NSLAB):
        xt = xt_pool.tile([128, KP, 2, SLAB], F8, tag="xt")
        if s == 0:
            # Split the first slab by k-pairs (contiguous in the tiled
            # layout) so the opening k-loop can start after the first
            # quarter arrives, pipelining through the rest.
            for q in range(4):
                nc.sync.dma_start(
                    xt[:, ts(q, KP // 4), :, :], xT[s, :, ts(q, KP // 4), :, :]
                )
        else:
            nc.sync.dma_start(xt[:], xT[s])
        an = anc_pool.tile([128, SUB, EMB], BF16, tag="an")
        nc.sync.dma_start(an[:], anc[s])
        wx4 = wx_pool.tile([128, SUB], BF16, tag="wx")
        last = s == NSLAB - 1
        for u in range(SUB):
            t = s * SUB + u
            h = h_pool.tile([128, ATT], F32, tag="h")
            for half in range(2):
                for kp in range(KP):
                    nc.tensor.matmul(
                        h[:, ts(half, ATT // 2)],
                        xt[:, kp, :, ts(u, 128)],
                        w1_sb[:, kp, :, ts(half, ATT // 2)],
                        start=(kp == 0), stop=(kp == KP - 1),
                        perf_mode=DR,
                    )
            if u == 2 and pending is not None:
                emit_step3_slab(pending)
            # In the final slab, trail its own step-3 work one tile behind the
            # matmul stream so only the last tile's score chain sits in the
            # kernel tail.
            if last and u >= 2:
                emit_step3_tile(wx4, an, s, u - 2)
            hb = hb_pool.tile([128, ATT], F32, tag="hb")
            nc.vector.scalar_tensor_tensor(
                hb[:], h[:], 1.0 / W1_SCALE, b1bc_sb[:],
                mybir.AluOpType.mult, mybir.AluOpType.add,
            )
            th = th_pool.tile([128, ATT], F32, tag="th")
            nc.scalar.activation(th[:], hb[:], mybir.ActivationFunctionType.Tanh)
            sc = sc_pool.tile([128, 1], F32, tag="sc")
            nc.vector.scalar_tensor_tensor(
                th[:], th[:], 1.0, w2b_sb[:],
                mybir.AluOpType.mult, mybir.AluOpType.mult, accum_out=sc[:],
            )
            nc.scalar.activation(wx4[:, u : u + 1], sc[:],
                                 mybir.ActivationFunctionType.Exp)
        pending = (wx4, an, s)
    wx4, an, s = pending
    emit_step3_tile(wx4, an, s, SUB - 2)
    emit_step3_tile(wx4, an, s, SUB - 1)
    nc.tensor.matmul(acc_se[:], wx4[:], ones_mv[:],
                     start=(s == 0), stop=True)

    out_sb = out_pool.tile([1, EMB], F32)
    se_sb = out_pool.tile([SUB, 1], F32)
    nc.vector.tensor_copy(out_sb[:, 0:ATT], acc0[:])
    nc.vector.tensor_copy(out_sb[:, ATT:EMB], acc1[:])
    nc.vector.tensor_copy(se_sb[:], acc_se[:])
    nc.sync.dma_start(partial_d[:], out_sb[:])
    nc.scalar.dma_start(wx_out_d[:], se_sb[:])


_nc_cache = None


def _get_nc():
    global _nc_cache
    if _nc_cache is None:
        nc = bacc.Bacc(
            "TRN2", target_bir_lowering=False, debug=False, num_devices=N_CORES
        )
        xT = nc.dram_tensor(
            "xT", [NSLAB, 128, KP, 2, SLAB], F8, kind="ExternalInput"
        ).ap()
        anc = nc.dram_tensor(
            "anc", [NSLAB, 128, SUB, EMB], BF16, kind="ExternalInput"
        ).ap()
        w1 = nc.dram_tensor("w1", [128, KP, 2, ATT], F8, kind="ExternalInput").ap()
        b1bc = nc.dram_tensor("b1bc", [128, ATT], BF16, kind="ExternalInput").ap()
        w2b = nc.dram_tensor("w2b", [128, ATT], F32, kind="ExternalInput").ap()
        partial = nc.dram_tensor("partial", [1, EMB], F32, kind="ExternalOutput").ap()
        wx_out = nc.dram_tensor("wx_out", [SUB, 1], F32, kind="ExternalOutput").ap()
        with tile.TileContext(nc) as tc, ExitStack() as ctx:
            _kernel_body(ctx, tc, xT, anc, w1, b1bc, w2b, partial, wx_out)
        nc.compile()
        _nc_cache = nc
    return _nc_cache


# Finite e4m3 value grid, for floor/ceil neighbor lookup in EF rounding.
_E4_GRID = None


def _e4_grid():
    global _E4_GRID
    if _E4_GRID is None:
        vals = np.arange(256, dtype=np.uint8).view(E4).astype(np.float32)
        _E4_GRID = np.unique(vals[np.isfinite(vals)])
    return _E4_GRID


def _ef_quantize_w1(W1, W2):
    """Quantize W1*W1_SCALE to e4m3 with per-row error-feedback rounding that
    nulls each row's quantization-residual projection onto W2 (the dominant
    fp8 error path into the softmax scores)."""
    grid = _e4_grid()
    W = (np.asarray(W1, dtype=np.float32) * W1_SCALE).astype(np.float32)
    w2vec = np.asarray(W2, dtype=np.float64).ravel()
    lo_i = np.clip(np.searchsorted(grid, W, side="right") - 1, 0, grid.size - 1)
    lo = grid[lo_i]
    hi = grid[np.clip(lo_i + 1, 0, grid.size - 1)]
    order = np.argsort(-np.abs(w2vec))
    q = np.empty_like(W)
    r = np.zeros(W.shape[0], dtype=np.float64)
    for t in order:
        dlo = (lo[:, t] - W[:, t]).astype(np.float64) * w2vec[t]
        dhi = (hi[:, t] - W[:, t]).astype(np.float64) * w2vec[t]
        pick_lo = np.abs(r + dlo) <= np.abs(r + dhi)
        q[:, t] = np.where(pick_lo, lo[:, t], hi[:, t])
        r += np.where(pick_lo, dlo, dhi)
    return q.astype(E4)


def _prep_core(c, leaves8, anc8, ancestors, shared):
    bf = ml_dtypes.bfloat16
    EC = EMB // 128  # feature chunks per source tensor
    xT = np.empty((NSLAB, 128, KP, 2, SLAB), dtype=E4)
    at = np.empty((NSLAB, 128, SUB, EMB), dtype=bf)
    # x feature f = kp*256 + j*128 + p maps to leaves[:, f] for f < EMB and
    # ancestors[:, f - EMB] otherwise; EMB = 4 k-pairs of 256.
    for s in range(NSLAB):
        rs = slice(c * R + s * SLAB, c * R + (s + 1) * SLAB)
        lv = leaves8[rs]      # [SLAB, EMB] fp8
        av = anc8[rs]
        xT[s, :, 0 : KP // 2] = lv.reshape(SLAB, KP // 2, 2, 128).transpose(3, 1, 2, 0)
        xT[s, :, KP // 2 : KP] = av.reshape(SLAB, KP // 2, 2, 128).transpose(3, 1, 2, 0)
        np.copyto(
            at[s], ancestors[rs].reshape(SUB, 128, EMB).transpose(1, 0, 2),
            casting="unsafe",
        )
    return {"xT": xT, "anc": at, **shared}


def kernel(leaves, ancestors, W1, b1, W2, b2, *, trace=False):
    global LAST_RESULTS
    nc = _get_nc()
    bf = ml_dtypes.bfloat16
    leaves = np.asarray(leaves, dtype=np.float32)
    ancestors = np.asarray(ancestors, dtype=np.float32)
    w1q = _ef_quantize_w1(W1, W2)  # [KF, ATT] e4m3, scaled by W1_SCALE
    shared = {
        "w1": np.ascontiguousarray(
            w1q.reshape(KP, 2, 128, ATT).transpose(2, 0, 1, 3)
        ),
        "b1bc": np.ascontiguousarray(
            np.broadcast_to(np.asarray(b1).astype(bf).reshape(1, ATT), (128, ATT))
        ),
        "w2b": np.ascontiguousarray(
            np.broadcast_to(
                np.asarray(W2, dtype=np.float32).reshape(1, ATT), (128, ATT)
            )
        ),
    }
    with ThreadPoolExecutor(max_workers=8) as ex:
        blocks = list(ex.map(lambda c: leaves[c * R : (c + 1) * R].astype(E4),
                             range(N_CORES)))
        leaves8 = np.concatenate(blocks)
        blocks = list(ex.map(lambda c: ancestors[c * R : (c + 1) * R].astype(E4),
                             range(N_CORES)))
        anc8 = np.concatenate(blocks)
        in_maps = list(
            ex.map(
                lambda c: _prep_core(c, leaves8, anc8, ancestors, shared),
                range(N_CORES),
            )
        )
    res = run_bass_kernel_spmd(
        nc, in_maps, core_ids=list(range(N_CORES)), trace=trace
    )
    LAST_RESULTS = res
    num = np.zeros(EMB, dtype=np.float64)
    den = 0.0
    for c in range(N_CORES):
        num += res.results[c]["partial"][0].astype(np.float64)
        den += res.results[c]["wx_out"].astype(np.float64).sum()
    return (num / den).astype(np.float32)


# revision 5
# speedup vs baseline: 1.6190x; 1.0874x over previous
"""DAG additive-attention kernel for 8 Trainium2 NeuronCores.

Reference computation (single fp32 graph):
    x = concat([leaves, ancestors], -1)            # [N, 2048]
    h = tanh(x @ W1 + b1)                          # [N, 512]
    scores = h @ W2 + b2                           # [N, 1]
    attn = softmax(scores, axis=0)
    out = attn.squeeze(1) @ ancestors              # [1024]

Distribution: shard N across the 8 cores. Softmax over N needs no on-device
collective: b2 is a constant shift (softmax-invariant, dropped), scores are
bounded (|h| <= 1 so |s| <= sum|W2| ~ 8), so exp() without max subtraction is
safe in fp32. Each core returns (sum_i exp(s_i) * ancestors_i, sum_i exp(s_i))
over its shard and the host combines in fp64.

The x @ W1 matmul runs in fp8 e4m3 with MatmulPerfMode.DoubleRow (K=256 per
instruction, 2x the bf16 FLOP rate). Precision holds because the dominant
fp8 error term is a bias proportional to (deltaW1_ancestor_rows @ W2) --
quantization noise in W1 correlates with the ancestors through the scores --
so the host quantizes W1*64 with per-row error-feedback rounding that drives
each row's residual projection onto W2 to ~0. The 1/64 descale rides the
(h + b1) DVE op as a scalar multiplier (scalar_tensor_tensor). The W2
multiply + score reduction fuse into one tensor_tensor_reduce DVE op.

Device pipeline per core (R = 8192 rows), processed in 512-row slabs whose
DMA reads are fully contiguous thanks to host-side pre-tiling: for each
128-row tile, 8 k-pair fp8 DoubleRow matmuls (stationary = transposed x pair
chunk, moving = W1 pair chunk, 2 halves of ATT) accumulate h in PSUM fp32;
DVE applies (h/64 + b1), ACT applies tanh, DVE (*W2, reduce) and ACT exp
produce the tile's softmax numerators, written into a per-slab [128, 4] grid
that then becomes the stationary operand of matmuls against the
natural-layout bf16 ancestors tile, accumulated into persistent PSUM
accumulators (plus one tiny matmul per slab against a ones vector for the
denominator). The step-3 matmul group trails the main matmul stream by one
slab so the PE never stalls on the score chain, and the PE clock gate is
pre-warmed with dummy matmuls during the initial DMA window.
"""

import sys

for _p in ("/opt/trn_rl_repo", "/opt/pypackages"):
    if _p not in sys.path:
        sys.path.append(_p)

from concurrent.futures import ThreadPoolExecutor
from contextlib import ExitStack

import ml_dtypes
import numpy as np

import concourse.tile as tile
from concourse import bacc, mybir
from concourse.bass import ts
from concourse.bass_utils import run_bass_kernel_spmd

N, EMB, ATT = 65536, 1024, 512
N_CORES = 8
R = N // N_CORES          # rows per core
KF = 2 * EMB              # contraction (feature) dim
KP = KF // 256            # k-pairs of 256 (fp8 DoubleRow)
SLAB = 512                # rows per DMA slab
NSLAB = R // SLAB
SUB = SLAB // 128         # 128-row subtiles per slab
NT = R // 128             # row tiles per core
BF16 = mybir.dt.bfloat16
F32 = mybir.dt.float32
F8 = mybir.dt.float8e4
E4 = ml_dtypes.float8_e4m3

W1_SCALE = 64.0           # W1 pre-quantization scale (fp8 normal range)

LAST_RESULTS = None       # BassKernelResults of the most recent run


WARM_MMS = 44  # PE clock-gate warmup matmuls emitted before the main loop


def _kernel_body(ctx, tc, xT, anc, w1, b1bc, w2b, partial_d, wx_out_d):
    nc = tc.nc
    DR = mybir.MatmulPerfMode.DoubleRow
    singles = ctx.enter_context(tc.tile_pool(name="singles", bufs=1))
    xt_pool = ctx.enter_context(tc.tile_pool(name="xt", bufs=4))
    anc_pool = ctx.enter_context(tc.tile_pool(name="anc", bufs=5))
    hb_pool = ctx.enter_context(tc.tile_pool(name="hb", bufs=3))
    th_pool = ctx.enter_context(tc.tile_pool(name="th", bufs=3))
    sc_pool = ctx.enter_context(tc.tile_pool(name="sc", bufs=4))
    wx_pool = ctx.enter_context(tc.tile_pool(name="wx", bufs=6))
    h_pool = ctx.enter_context(tc.tile_pool(name="hps", bufs=4, space="PSUM"))
    acc_pool = ctx.enter_context(tc.tile_pool(name="acc", bufs=1, space="PSUM"))
    out_pool = ctx.enter_context(tc.tile_pool(name="outs", bufs=1))

    # Weights / constants, resident for the whole kernel. Issued on the
    # scalar-engine HWDGE ring so they overlap the first xT slab (sync ring).
    # All DRAM operands arrive pre-tiled from the host so every transfer
    # below is a fully contiguous read.
    w1_sb = singles.tile([128, KP, 2, ATT], F8)
    for q in range(4):
        nc.scalar.dma_start(w1_sb[:, ts(q, KP // 4), :, :], w1[:, ts(q, KP // 4), :, :])
    b1bc_sb = singles.tile([128, ATT], BF16)
    nc.scalar.dma_start(b1bc_sb[:], b1bc[:])
    w2b_sb = singles.tile([128, ATT], BF16)
    nc.scalar.dma_start(w2b_sb[:], w2b[:])
    ones_mv = singles.tile([128, 1], BF16)
    nc.vector.memset(ones_mv[:], 1.0)

    # Warm the PE clock gate during the initial DMA window: dependency-free
    # matmuls on a zeroed tile keep TensorE busy >3.4us so the HAM throttle is
    # released right around the time the first slab and weights arrive.
    if WARM_MMS:
        warm_sb = singles.tile([128, 128], BF16)
        nc.vector.memset(warm_sb[:], 0.0)
        warm_ps = h_pool.tile([128, 128], F32, tag="h")
        for _ in range(WARM_MMS):
            nc.tensor.matmul(
                warm_ps[:], warm_sb[:], warm_sb[:], start=True, stop=True
            )

    # Persistent PSUM accumulators: weighted ancestor sums (acc0/acc1) and the
    # per-subtile-phase sums of the exp weights (acc_se, reduced on host).
    acc0 = acc_pool.tile([1, ATT], F32, tag="acc0")
    acc1 = acc_pool.tile([1, ATT], F32, tag="acc1")
    acc_se = acc_pool.tile([SUB, 1], F32, tag="accse")

    def emit_step3_tile(wx4, an_t, s, u):
        t = s * SUB + u
        st, sp = (t == 0), (t == NT - 1)
        nc.tensor.matmul(acc0[:], wx4[:, u : u + 1], an_t[:, u, 0:ATT],
                         start=st, stop=sp)
        nc.tensor.matmul(acc1[:], wx4[:, u : u + 1], an_t[:, u, ATT:EMB],
                         start=st, stop=sp)

    def emit_step3_slab(p):
        wx4, an_t, s = p
        for u in range(SUB):
            emit_step3_tile(wx4, an_t, s, u)
        nc.tensor.matmul(acc_se[:], wx4[:], ones_mv[:],
                         start=(s == 0), stop=(s == NSLAB - 1))

    pending = None
    for s in range(NSLAB):
        xt = xt_pool.tile([128, KP, 2, SLAB], F8, tag="xt")
        if s == 0:
            # Split the first slab by k-pairs (contiguous in the tiled
            # layout) so the opening k-loop can start after the first
            # quarter arrives, pipelining through the rest.
            for q in range(4):
                nc.sync.dma_start(
                    xt[:, ts(q, KP // 4), :, :], xT[s, :, ts(q, KP // 4), :, :]
                )
        else:
            nc.sync.dma_start(xt[:], xT[s])
        an = anc_pool.tile([128, SUB, EMB], BF16, tag="an")
        nc.sync.dma_start(an[:], anc[s])
        wx4 = wx_pool.tile([128, SUB], BF16, tag="wx")
        last = s == NSLAB - 1
        for u in range(SUB):
            t = s * SUB + u
            h = h_pool.tile([128, ATT], F32, tag="h")
            for half in range(2):
                for kp in range(KP):
                    nc.tensor.matmul(
                        h[:, ts(half, ATT // 2)],
                        xt[:, kp, :, ts(u, 128)],
                        w1_sb[:, kp, :, ts(half, ATT // 2)],
                        start=(kp == 0), stop=(kp == KP - 1),
                        perf_mode=DR,
                    )
            if u == 3 and pending is not None:
                emit_step3_slab(pending)
            # In the final slab, trail its own step-3 work one tile behind the
            # matmul stream so only the last tile's score chain sits in the
            # kernel tail.
            if last and u >= 2:
                emit_step3_tile(wx4, an, s, u - 2)
            hb = hb_pool.tile([128, ATT], F32, tag="hb")
            nc.vector.scalar_tensor_tensor(
                hb[:], h[:], 1.0 / W1_SCALE, b1bc_sb[:],
                mybir.AluOpType.mult, mybir.AluOpType.add,
            )
            th = th_pool.tile([128, ATT], BF16, tag="th")
            nc.scalar.activation(th[:], hb[:], mybir.ActivationFunctionType.Tanh)
            sc = sc_pool.tile([128, 1], F32, tag="sc")
            nc.vector.scalar_tensor_tensor(
                th[:], th[:], 1.0, w2b_sb[:],
                mybir.AluOpType.mult, mybir.AluOpType.mult, accum_out=sc[:],
            )
            nc.scalar.activation(wx4[:, u : u + 1], sc[:],
                                 mybir.ActivationFunctionType.Exp)
        pending = (wx4, an, s)
    wx4, an, s = pending
    emit_step3_tile(wx4, an, s, SUB - 2)
    emit_step3_tile(wx4, an, s, SUB - 1)
    nc.tensor.matmul(acc_se[:], wx4[:], ones_mv[:],
                     start=(s == 0), stop=True)

    out_sb = out_pool.tile([1, EMB], F32)
    se_sb = out_pool.tile([SUB, 1], F32)
    nc.vector.tensor_copy(out_sb[:, 0:ATT], acc0[:])
    nc.vector.tensor_copy(out_sb[:, ATT:EMB], acc1[:])
    nc.vector.tensor_copy(se_sb[:], acc_se[:])
    nc.sync.dma_start(partial_d[:], out_sb[:])
    nc.scalar.dma_start(wx_out_d[:], se_sb[:])


_nc_cache = None


def _get_nc():
    global _nc_cache
    if _nc_cache is None:
        nc = bacc.Bacc(
            "TRN2", target_bir_lowering=False, debug=False, num_devices=N_CORES
        )
        xT = nc.dram_tensor(
            "xT", [NSLAB, 128, KP, 2, SLAB], F8, kind="ExternalInput"
        ).ap()
        anc = nc.dram_tensor(
            "anc", [NSLAB, 128, SUB, EMB], BF16, kind="ExternalInput"
        ).ap()
        w1 = nc.dram_tensor("w1", [128, KP, 2, ATT], F8, kind="ExternalInput").ap()
        b1bc = nc.dram_tensor("b1bc", [128, ATT], BF16, kind="ExternalInput").ap()
        w2b = nc.dram_tensor("w2b", [128, ATT], BF16, kind="ExternalInput").ap()
        partial = nc.dram_tensor("partial", [1, EMB], F32, kind="ExternalOutput").ap()
        wx_out = nc.dram_tensor("wx_out", [SUB, 1], F32, kind="ExternalOutput").ap()
        with tile.TileContext(nc) as tc, ExitStack() as ctx:
            _kernel_body(ctx, tc, xT, anc, w1, b1bc, w2b, partial, wx_out)
        nc.compile()
        _nc_cache = nc
    return _nc_cache


# Finite e4m3 value grid, for floor/ceil neighbor lookup in EF rounding.
_E4_GRID = None


def _e4_grid():
    global _E4_GRID
    if _E4_GRID is None:
        vals = np.arange(256, dtype=np.uint8).view(E4).astype(np.float32)
        _E4_GRID = np.unique(vals[np.isfinite(vals)])
    return _E4_GRID


def _ef_quantize_w1(W1, W2):
    """Quantize W1*W1_SCALE to e4m3 with per-row error-feedback rounding that
    nulls each row's quantization-residual projection onto W2 (the dominant
    fp8 error path into the softmax scores)."""
    grid = _e4_grid()
    W = (np.asarray(W1, dtype=np.float32) * W1_SCALE).astype(np.float32)
    w2vec = np.asarray(W2, dtype=np.float64).ravel()
    lo_i = np.clip(np.searchsorted(grid, W, side="right") - 1, 0, grid.size - 1)
    lo = grid[lo_i]
    hi = grid[np.clip(lo_i + 1, 0, grid.size - 1)]
    order = np.argsort(-np.abs(w2vec))
    q = np.empty_like(W)
    r = np.zeros(W.shape[0], dtype=np.float64)
    for t in order:
        dlo = (lo[:, t] - W[:, t]).astype(np.float64) * w2vec[t]
        dhi = (hi[:, t] - W[:, t]).astype(np.float64) * w2vec[t]
        pick_lo = np.abs(r + dlo) <= np.abs(r + dhi)
        q[:, t] = np.where(pick_lo, lo[:, t], hi[:, t])
        r += np.where(pick_lo, dlo, dhi)
    return q.astype(E4)


def _prep_core(c, leaves8, anc8, ancestors, shared):
    bf = ml_dtypes.bfloat16
    EC = EMB // 128  # feature chunks per source tensor
    xT = np.empty((NSLAB, 128, KP, 2, SLAB), dtype=E4)
    at = np.empty((NSLAB, 128, SUB, EMB), dtype=bf)
    # x feature f = kp*256 + j*128 + p maps to leaves[:, f] for f < EMB and
    # ancestors[:, f - EMB] otherwise; EMB = 4 k-pairs of 256.
    for s in range(NSLAB):
        rs = slice(c * R + s * SLAB, c * R + (s + 1) * SLAB)
        lv = leaves8[rs]      # [SLAB, EMB] fp8
        av = anc8[rs]
        xT[s, :, 0 : KP // 2] = lv.reshape(SLAB, KP // 2, 2, 128).transpose(3, 1, 2, 0)
        xT[s, :, KP // 2 : KP] = av.reshape(SLAB, KP // 2, 2, 128).transpose(3, 1, 2, 0)
        np.copyto(
            at[s], ancestors[rs].reshape(SUB, 128, EMB).transpose(1, 0, 2),
            casting="unsafe",
        )
    return {"xT": xT, "anc": at, **shared}


def kernel(leaves, ancestors, W1, b1, W2, b2, *, trace=False):
    global LAST_RESULTS
    nc = _get_nc()
    bf = ml_dtypes.bfloat16
    leaves = np.asarray(leaves, dtype=np.float32)
    ancestors = np.asarray(ancestors, dtype=np.float32)
    w1q = _ef_quantize_w1(W1, W2)  # [KF, ATT] e4m3, scaled by W1_SCALE
    shared = {
        "w1": np.ascontiguousarray(
            w1q.reshape(KP, 2, 128, ATT).transpose(2, 0, 1, 3)
        ),
        "b1bc": np.ascontiguousarray(
            np.broadcast_to(np.asarray(b1).astype(bf).reshape(1, ATT), (128, ATT))
        ),
        "w2b": np.ascontiguousarray(
            np.broadcast_to(
                np.asarray(W2, dtype=np.float32).astype(bf).reshape(1, ATT),
                (128, ATT),
            )
        ),
    }
    with ThreadPoolExecutor(max_workers=8) as ex:
        blocks = list(ex.map(lambda c: leaves[c * R : (c + 1) * R].astype(E4),
                             range(N_CORES)))
        leaves8 = np.concatenate(blocks)
        blocks = list(ex.map(lambda c: ancestors[c * R : (c + 1) * R].astype(E4),
                             range(N_CORES)))
        anc8 = np.concatenate(blocks)
        in_maps = list(
            ex.map(
                lambda c: _prep_core(c, leaves8, anc8, ancestors, shared),
                range(N_CORES),
            )
        )
    res = run_bass_kernel_spmd(
        nc, in_maps, core_ids=list(range(N_CORES)), trace=trace
    )
    LAST_RESULTS = res
    num = np.zeros(EMB, dtype=np.float64)
    den = 0.0
    for c in range(N_CORES):
        num += res.results[c]["partial"][0].astype(np.float64)
        den += res.results[c]["wx_out"].astype(np.float64).sum()
    return (num / den).astype(np.float32)
